# revision 10
# baseline (speedup 1.0000x reference)
"""Equivariant block-diagonal linear (128x0e+128x1o+64x2e+32x3o) on 8 trn2 cores.

Strategy:
  - Data-parallel: x [50000, 1056] row-sharded into 8x [6250, 1056].
  - Per irrep r, the op is out[n, w*d+j] = sum_u w_r[u,w] * x[n, u*d+i] delta_ij,
    i.e. a dense matmul with WD_r = kron(w_r, I_d)  [mul*d, mul*d], built on host.
  - Per 128-node subtile: PE-transpose irrep-aligned f-chunks of x ([n,f] -> [f,n]
    in PSUM), copy to SBUF, then matmul with xT as the *stationary* operand and
    WD rows as the *moving* operand: out[n, g] = sum_f xT[f, n] * WD[f, g].
    The output lands in natural [node, feature] layout -> contiguous DMA out.
"""

import os
from contextlib import ExitStack

import numpy as np

import concourse.bass as bass
import concourse.tile as tile
from concourse import bacc, mybir
from concourse.bass_utils import run_bass_kernel_spmd

N_NODES = 50000
DIM = 1056
N_CORES = 8
SHARD = N_NODES // N_CORES  # 6250
P = 128  # nodes per subtile
N_SUB = (SHARD + P - 1) // P  # 49 (last has 106 rows)
N_SUB_RUN = int(os.environ.get("KERNEL_NSUB", str(N_SUB)))

IRREPS = [(128, 0), (128, 1), (64, 2), (32, 3)]
# per-irrep feature offset and span (= mul * (2l+1))
G_OFF = [0, 128, 512, 832]
G_SPAN = [128, 384, 320, 224]
# f-chunks (irrep-aligned, each <=128 wide): (global_off, local_off, width)
F_CHUNKS = [
    [(0, 0, 128)],
    [(128, 0, 128), (256, 128, 128), (384, 256, 128)],
    [(512, 0, 128), (640, 128, 128), (768, 256, 64)],
    [(832, 0, 128), (960, 128, 96)],
]

USE_FP32R = os.environ.get("KERNEL_FP32R", "1") == "1"
# fp32r matmul runs 1 cyc/row only when moving free dim >= 256 -> pad g-spans
G_PAD = [256, 384, 320, 256] if USE_FP32R else list(G_SPAN)

_cache = {}


def _build():
    key = ("prog", USE_FP32R, N_SUB_RUN)
    if key in _cache:
        return _cache[key]
    f32 = mybir.dt.float32
    f32r = mybir.dt.float32r
    nc = bacc.Bacc(
        "TRN2", target_bir_lowering=False, debug=False, num_devices=N_CORES
    )
    x_d = nc.dram_tensor("x", [SHARD, DIM], f32, kind="ExternalInput")
    wd_d = [
        nc.dram_tensor(f"wd{r}", [G_SPAN[r], G_PAD[r]], f32, kind="ExternalInput")
        for r in range(4)
    ]
    out_d = nc.dram_tensor("out", [SHARD, DIM], f32, kind="ExternalOutput")
    ident_d = nc.inline_tensor(np.eye(P, dtype=np.float32), name="ident")

    with ExitStack() as ctx:
        tc = ctx.enter_context(tile.TileContext(nc))
        wpool = ctx.enter_context(tc.tile_pool(name="w", bufs=1))
        xpool = ctx.enter_context(tc.tile_pool(name="xin", bufs=8))
        opool = ctx.enter_context(tc.tile_pool(name="oout", bufs=8))
        xtpool = ctx.enter_context(tc.tile_pool(name="xt", bufs=12))
        ps_t = ctx.enter_context(tc.tile_pool(name="ps_t", bufs=3, space="PSUM"))
        ps_o = ctx.enter_context(tc.tile_pool(name="ps_o", bufs=4, space="PSUM"))

        ident = wpool.tile([P, P], f32, tag="ident")
        nc.sync.dma_start(ident[:], ident_d[:])
        wsb = {}
        wdt = f32r if USE_FP32R else f32
        for r in range(4):
            for j, (_, fl, fw) in enumerate(F_CHUNKS[r]):
                t = wpool.tile([fw, G_PAD[r]], wdt, tag=f"w{r}_{j}")
                if USE_FP32R:
                    stg = wpool.tile([fw, G_PAD[r]], f32, tag=f"wstg{r}_{j}")
                    nc.sync.dma_start(stg[:], wd_d[r][fl : fl + fw, :])
                    nc.vector.tensor_copy(t[:], stg[:])
                else:
                    nc.sync.dma_start(t[:], wd_d[r][fl : fl + fw, :])
                wsb[(r, j)] = t

        ci = 0
        for s in range(N_SUB_RUN):
            rows = min(P, SHARD - s * P)
            xt_in = xpool.tile([P, DIM], f32, tag="x")
            half = DIM // 2
            nc.sync.dma_start(xt_in[:rows, :half], x_d[s * P : s * P + rows, :half])
            nc.gpsimd.dma_start(
                xt_in[:rows, half:], x_d[s * P : s * P + rows, half:]
            )
            out_t = opool.tile([P, DIM], f32, tag="o")
            for r in range(4):
                po = ps_o.tile([P, G_PAD[r]], f32, tag="po")
                nchunks = len(F_CHUNKS[r])
                for j, (fg, fl, fw) in enumerate(F_CHUNKS[r]):
                    pt = ps_t.tile([P, P], f32, tag="pt")
                    nc.tensor.transpose(
                        pt[:fw, :rows],
                        xt_in[:rows, fg : fg + fw],
                        ident[:rows, :rows],
                    )
                    xt_sb = xtpool.tile([P, P], wdt, tag="xt")
                    if ci % 2 == 0:
                        nc.vector.tensor_copy(xt_sb[:fw, :rows], pt[:fw, :rows])
                    else:
                        nc.scalar.copy(xt_sb[:fw, :rows], pt[:fw, :rows])
                    ci += 1
                    lhs = xt_sb[:fw, :rows]
                    rhs = wsb[(r, j)][:, :]
                    nc.tensor.matmul(
                        po[:rows, :],
                        lhs,
                        rhs,
                        start=(j == 0),
                        stop=(j == nchunks - 1),
                    )
                if ci % 2 == 0:
                    nc.vector.tensor_copy(
                        out_t[:rows, G_OFF[r] : G_OFF[r] + G_SPAN[r]],
                        po[:rows, : G_SPAN[r]],
                    )
                else:
                    nc.scalar.copy(
                        out_t[:rows, G_OFF[r] : G_OFF[r] + G_SPAN[r]],
                        po[:rows, : G_SPAN[r]],
                    )
                ci += 1
            nc.sync.dma_start(
                out_d[s * P : s * P + rows, :half], out_t[:rows, :half]
            )
            nc.gpsimd.dma_start(
                out_d[s * P : s * P + rows, half:], out_t[:rows, half:]
            )

    nc.compile()
    _cache[key] = nc
    return nc



# ---------------------------------------------------------------------------
# HT mode: host-transposed layout. Device sees xT [1056, SHARD] and writes
# outT [1056, SHARD]. W blocks are stationary (LDW amortized over node
# groups); xT chunks stream as the moving operand with N=512.
# ---------------------------------------------------------------------------
NG = 512  # nodes per group
N_GRP = (SHARD + NG - 1) // NG  # 13 (last = 106)
NBLK = 4  # groups per W-residency block

# all 128-aligned-ish g-chunks (same 9 chunks as F_CHUNKS, flat)
CHUNKS9 = [(r, fg, fl, fw) for r in range(4) for (fg, fl, fw) in F_CHUNKS[r]]
# W blocks keyed (r, jf, jg): [f-chunk jf, g-chunk jg] of irrep r
W_BLOCKS = []
for r in range(4):
    for jf, (_, fl, fw) in enumerate(F_CHUNKS[r]):
        for jg, (_, gl, gw) in enumerate(F_CHUNKS[r]):
            W_BLOCKS.append((r, jf, jg, fl, fw, gl, gw))


def _build_ht():
    key = ("ht", USE_FP32R)
    if key in _cache:
        return _cache[key]
    f32 = mybir.dt.float32
    f32r = mybir.dt.float32r
    mmdt = f32r if USE_FP32R else f32
    nc = bacc.Bacc(
        "TRN2", target_bir_lowering=False, debug=False, num_devices=N_CORES
    )
    xt_d = nc.dram_tensor("xt", [DIM, SHARD], f32, kind="ExternalInput")
    wd_d = [
        nc.dram_tensor(f"wd{r}", [G_SPAN[r], G_SPAN[r]], f32, kind="ExternalInput")
        for r in range(4)
    ]
    out_d = nc.dram_tensor("outt", [DIM, SHARD], f32, kind="ExternalOutput")

    with ExitStack() as ctx:
        tc = ctx.enter_context(tile.TileContext(nc))
        wpool = ctx.enter_context(tc.tile_pool(name="w", bufs=1))
        xpool = ctx.enter_context(tc.tile_pool(name="xin", bufs=1))
        opool = ctx.enter_context(tc.tile_pool(name="oout", bufs=10))
        ps_o = ctx.enter_context(tc.tile_pool(name="ps_o", bufs=6, space="PSUM"))

        # resident W blocks (rounded to f32r via staging copy when needed)
        wsb = {}
        for bi, (r, jf, jg, fl, fw, gl, gw) in enumerate(W_BLOCKS):
            t = wpool.tile([fw, gw], mmdt, tag=f"wb{bi}")
            if USE_FP32R:
                stg = wpool.tile([fw, gw], f32, tag=f"wstg{bi}")
                nc.sync.dma_start(stg[:], wd_d[r][fl : fl + fw, gl : gl + gw])
                nc.vector.tensor_copy(t[:], stg[:])
            else:
                nc.sync.dma_start(t[:], wd_d[r][fl : fl + fw, gl : gl + gw])
            wsb[(r, jf, jg)] = t

        dma_engines = [nc.sync, nc.gpsimd, nc.scalar]
        for blk0 in range(0, N_GRP, NBLK):
            grps = list(range(blk0, min(blk0 + NBLK, N_GRP)))
            # load xT chunks for these groups
            xts = {}
            for gi, g in enumerate(grps):
                cols = min(NG, SHARD - g * NG)
                for c9, (r, fg, fl, fw) in enumerate(CHUNKS9):
                    xin = xpool.tile([P, NG], f32r if USE_FP32R else f32, tag=f"xt{gi}_{c9}")
                    eng = dma_engines[(gi + c9) % len(dma_engines)]
                    eng.dma_start(
                        xin[:fw, :cols],
                        xt_d[fg : fg + fw, g * NG : g * NG + cols].bitcast(xin.dtype)
                        if USE_FP32R
                        else xt_d[fg : fg + fw, g * NG : g * NG + cols],
                    )
                    xts[(gi, c9)] = xin
            # chunk index within irrep -> global chunk9 index
            base9 = [0, 1, 4, 7]
            cc = 0
            for r in range(4):
                nch = len(F_CHUNKS[r])
                for jg in range(nch):
                    _, gl, gw = F_CHUNKS[r][jg]
                    goff = G_OFF[r] + gl
                    pos = []
                    for _gi in range(len(grps)):
                        po = ps_o.tile([P, NG], f32, tag="po")
                        pos.append(po)
                    for jf in range(nch):
                        blk = wsb[(r, jf, jg)]
                        for gi, g in enumerate(grps):
                            cols = min(NG, SHARD - g * NG)
                            c9 = base9[r] + jf
                            nc.tensor.matmul(
                                pos[gi][:gw, :cols],
                                blk[:, :],
                                xts[(gi, c9)][: blk.shape[0], :cols],
                                start=(jf == 0),
                                stop=(jf == nch - 1),
                            )
                    for gi, g in enumerate(grps):
                        cols = min(NG, SHARD - g * NG)
                        ot = opool.tile([P, NG], f32, tag="ot")
                        cc += 1
                        if (cc + gi) % 2 == 0:
                            nc.vector.tensor_copy(
                                ot[:gw, :cols], pos[gi][:gw, :cols]
                            )
                        else:
                            nc.scalar.copy(ot[:gw, :cols], pos[gi][:gw, :cols])
                        eng = dma_engines[(gi + jg) % len(dma_engines)]
                        eng.dma_start(
                            out_d[goff : goff + gw, g * NG : g * NG + cols],
                            ot[:gw, :cols],
                        )

    nc.compile()
    _cache[key] = nc
    return nc


def _dense_weights(ws):
    out = []
    for r, (mul, l) in enumerate(IRREPS):
        d = 2 * l + 1
        wd = np.kron(np.asarray(ws[r], dtype=np.float32), np.eye(d, dtype=np.float32))
        if G_PAD[r] != wd.shape[1]:
            wd = np.pad(wd, ((0, 0), (0, G_PAD[r] - wd.shape[1])))
        out.append(np.ascontiguousarray(wd, dtype=np.float32))
    return out


last_result = None  # BassKernelResults of the most recent run (for profiling)


MODE = os.environ.get("KERNEL_MODE", "ht")


def kernel(x, w0, w1, w2, w3):
    global last_result
    x = np.asarray(x, dtype=np.float32)
    wds = _dense_weights([w0, w1, w2, w3])
    trace = os.environ.get("KERNEL_TRACE", "0") == "1"
    if MODE == "ht":
        nc = _build_ht()
        in_maps = []
        for c in range(N_CORES):
            m = {"xt": np.ascontiguousarray(x[c * SHARD : (c + 1) * SHARD].T)}
            for r in range(4):
                m[f"wd{r}"] = wds[r][:, : G_SPAN[r]]
            in_maps.append(m)
        last_result = run_bass_kernel_spmd(
            nc, in_maps, core_ids=list(range(N_CORES)), trace=trace
        )
        return np.ascontiguousarray(
            np.concatenate([r["outt"].T for r in last_result.results], axis=0)
        )
    nc = _build()
    x = np.ascontiguousarray(x)
    in_maps = []
    for c in range(N_CORES):
        m = {"x": x[c * SHARD : (c + 1) * SHARD]}
        for r in range(4):
            m[f"wd{r}"] = wds[r]
        in_maps.append(m)
    last_result = run_bass_kernel_spmd(
        nc, in_maps, core_ids=list(range(N_CORES)), trace=trace
    )
    return np.concatenate([r["out"] for r in last_result.results], axis=0)
